# revision 13
# baseline (speedup 1.0000x reference)
"""Trainium2 Bass kernel for nn_Averager (pooling, 3-level box-average).

Math (verified vs reference): per sample, with input x[n, i, c] where
n = (n5 n4 n3 n2 n1 n0) base-4 digits, c = (c2 c1 c0) base-4 digits:
  out[:, :, 0, :] = x[:, :, 0, :]
  out1[n, c] = E[n4, n2, c2, c0, n0, c1],
      E[r5, r4, r3, r0; g2, g1] = mean over (n2, n1, c0) of x1
  out2[n, c] = G[c2, c1, c0],
      G[p, q, r] = mean over (n4, n3, n1, n0, c1, c0) of x2 with
      (n5, c2in, n2) = (p, q, r)

Sharding: data-parallel over batch, 4 samples per core on 8 cores,
processed as 2 groups of 2 samples.

Layout (pair-contiguous): SBUF partition p = b*64 + n//64 =
(b, n5, n4, n3); free j = n % 64 = 16*n2 + 4*n1 + n0, row (i, c).
A 6MB group is contiguous in DRAM and per-partition contiguous in SBUF.
All reductions are lane-local (reduced digits n2, n1, n0, c1, c0 all
live in the free dim); the PE selector matmuls only route/broadcast
E (16 matmuls) and reduce+broadcast G (4 matmuls) across partitions.
Selectors and the matmul moving operands (A, A2 partial sums) are bf16:
selector values 2^-6 / 2^-12 are exact in bf16, the partial sums round
at 2^-9 relative (measured 4e-4 headline vs the 2e-2 gate); bf16 makes
PE ~8x faster than fp32 (33us -> 9us busy) and halves the selector
load.  PE reads the selector tile straight from its landing buffer
(LoadWeights carries the one DMA-sem wait) - re-copying through DVE
head-blocked every DVE op behind a constant that trickles in behind
the 12.6MB input stream.

Output is assembled IN-PLACE into the input tile (level regions are
dead once the partial reductions are done), making the out tile
byte-identical to the DRAM output layout, so each store is a
fully-contiguous 3MB DMA (48KB runs per partition).  Writing levels
separately would emit 256B packets (the (N, 3, 64) layout interleaves
levels every 256B), below the 512B SDMA line-rate threshold and
measured 4x slower.

DMA schedule: measured direction-mixing law on the SDMA engines: reads
alone ~425 GB/s (fabric), writes alone ~390, but concurrent read+write
queues collapse to ~170-320 aggregate - below the ~407 harmonic
break-even, so serial read-then-write phases beat any overlap.  Two
concurrent write queues also degrade (~318 vs 390).  Hence ALL x loads
and ALL stores ride the ONE SP ring in program order (loads first -
ring FIFO enforces the phase split and keeps a single queue active),
and the ACT ring carries only the selector load (read+read mixing is
penalty-free, measured 413+ GB/s during the load phase).  Every store
trigger releases >=8us before the ring FIFO reaches it, so the whole
compute chain is off the critical path and the makespan is just
preamble + reads@425 + writes@390 + teardown.

Engine split: DVE runs the reduction ladders and ALL E-evacs (DVE
copies are cheaper than ACT's and this keeps the ACT ring free to fire
stores the moment their half is assembled); ACT only broadcasts the
64-float G row into the level-2 rows and triggers its stores in
program order.  The DVE ladder is ordered so everything that needs
only rows j<32 (u0, t4 first half) runs while the second half of the
load is still in flight, and A2 lands before A so the 4-matmul S2
stage (feeding the G broadcasts) clears PE before the 16-matmul S1
stage (feeding the evacs).
"""

import numpy as np

N_CORES = 8
B_FULL = 32
B_CORE = B_FULL // N_CORES  # 4
N = 4096
LVL = 3
C = 64


def _make_selectors():
    """Routing selectors, pair layout: k = 64*b + 16*k5 + 4*k4 + k3.

    S1 block (n2o, c2o), 16 blocks:
        S1[k, m] = 1/64   iff b(k)==b(m), k5==m4, k4==n2o, k3==c2o
    S2 block (c2o), 4 blocks:
        S2[k, m] = 1/4096 iff b(k)==b(m), k5==c2o
    """
    k = np.arange(128)
    b, k5, k4, k3 = k >> 6, (k >> 4) & 3, (k >> 2) & 3, k & 3
    m = np.arange(128)
    bm, m4 = m >> 6, (m >> 2) & 3
    S1 = np.zeros((128, 16, 128), np.float32)
    S2 = np.zeros((128, 4, 128), np.float32)
    for n2o in range(4):
        for c2o in range(4):
            S1[:, n2o * 4 + c2o, :] = (
                (b[:, None] == bm[None, :])
                & (k5[:, None] == m4[None, :])
                & (k4[:, None] == n2o)
                & (k3[:, None] == c2o)
            ).astype(np.float32) / 64.0
    for c2o in range(4):
        S2[:, c2o, :] = (
            (b[:, None] == bm[None, :]) & (k5[:, None] == c2o)
        ).astype(np.float32) / 4096.0
    return (
        np.ascontiguousarray(S1.reshape(128, 2048)),
        np.ascontiguousarray(S2.reshape(128, 512)),
    )


def _build_nc():
    import concourse.bass as bass
    import concourse.tile as tile
    from concourse import mybir

    dt = mybir.dt.float32
    bf = mybir.dt.bfloat16
    X = mybir.AxisListType.X
    ADD = mybir.AluOpType.add

    from concourse import bacc
    nc = bacc.Bacc()
    x = nc.declare_dram_parameter("x", [B_CORE, N, LVL, C], dt, isOutput=False)
    s12 = nc.declare_dram_parameter("s12", [128, 2560], bf, isOutput=False)
    out = nc.declare_dram_parameter("out", [B_CORE, N, LVL, C], dt, isOutput=True)

    with tile.TileContext(nc) as tc:
        with (
            tc.tile_pool(name="consts", bufs=1) as cpool,
            tc.tile_pool(name="xin", bufs=2) as xpool,
            tc.tile_pool(name="tmp", bufs=1) as tpool,
            tc.tile_pool(name="psum", bufs=2, space="PSUM") as ppool,
        ):
            # selector load heads the ACT ring (stores queue there much
            # later); PE reads it in place, no re-copy
            s12sb = cpool.tile([128, 2560], bf, tag="s12")
            nc.scalar.dma_start(s12sb[:], s12[:])
            s1sb = s12sb[:, 0:2048]
            s2sb = s12sb[:, 2048:2560]

            stores = []
            for g in range(B_CORE // 2):
                bs = slice(2 * g, 2 * g + 2)
                xt = xpool.tile([128, 12288], dt, tag="xt")
                # split the 6MB load so the ladder's j<32 work starts
                # after the first half lands
                xsrc = x[bs].rearrange("b (ph j) i c -> (b ph) (j i c)", ph=64)
                nc.sync.dma_start(xt[:, 0:6144], xsrc[:, 0:6144])
                nc.sync.dma_start(xt[:, 6144:12288], xsrc[:, 6144:12288])
                xtv = xt[:].rearrange(
                    "p (j i c) -> p j i c", j=64, i=3, c=64
                )

                # ---- DVE ladder, j<32 part: u0 and the L2 t4 half ----
                v = xt[:].rearrange(
                    "p (n2 n1 n0 i c) -> p n2 n1 n0 i c",
                    n2=4, n1=4, n0=4, i=3, c=64,
                )
                xw = xt[:].rearrange(
                    "p (j i c2 cc) -> p j i c2 cc", j=64, i=3, c2=4, cc=16
                )
                u0 = tpool.tile([128, 1024], dt, tag="u0")
                nc.vector.tensor_add(
                    u0[:].rearrange("p (n1 n0 c) -> p n1 n0 c", n1=4, n0=4, c=64),
                    v[:, 0, :, :, 1, :], v[:, 1, :, :, 1, :],
                )
                t4 = tpool.tile([128, 256], dt, tag="t4")
                t4v = t4[:].rearrange("p (j c2) -> p j c2", j=64, c2=4)
                nc.vector.tensor_reduce(
                    t4v[:, 0:32, :], xw[:, 0:32, 2, :, :], axis=X, op=ADD,
                )
                # ---- j>=32 part ----
                u1 = tpool.tile([128, 1024], dt, tag="u1")
                nc.vector.tensor_add(
                    u1[:].rearrange("p (n1 n0 c) -> p n1 n0 c", n1=4, n0=4, c=64),
                    v[:, 2, :, :, 1, :], v[:, 3, :, :, 1, :],
                )
                w = tpool.tile([128, 1024], dt, tag="w")
                nc.vector.tensor_add(w[:], u0[:], u1[:])
                h1 = tpool.tile([128, 512], dt, tag="h1")
                nc.vector.tensor_add(h1[:], w[:, 0:512], w[:, 512:1024])
                h2 = tpool.tile([128, 256], dt, tag="h2")
                nc.vector.tensor_add(h2[:], h1[:, 0:256], h1[:, 256:512])
                nc.vector.tensor_reduce(
                    t4v[:, 32:64, :], xw[:, 32:64, 2, :, :], axis=X, op=ADD,
                )
                # A2 before A: S2 (4 matmuls, feeds the G broadcasts)
                # clears PE before the 16-matmul S1 stage
                A2 = tpool.tile([128, 16], bf, tag="A2")
                with nc.allow_low_precision(
                    reason="bf16 partial sums round at 2^-9, gate is 2e-2"
                ):
                    nc.vector.tensor_reduce(
                        A2[:].rearrange("p (c2 n2) -> p n2 c2", c2=4, n2=4),
                        t4[:].rearrange(
                            "p (n2 nn c2) -> p n2 c2 nn", n2=4, nn=16, c2=4
                        ),
                        axis=X, op=ADD,
                    )
                    # reduce c0, write A with free = 16*c2 + 4*c1 + n0
                    A = tpool.tile([128, 64], bf, tag="A")
                    nc.vector.tensor_reduce(
                        A[:].rearrange(
                            "p (c2 c1 n0) -> p n0 c2 c1", c2=4, c1=4, n0=4
                        ),
                        h2[:].rearrange(
                            "p (n0 c2 c1 c0) -> p n0 c2 c1 c0",
                            n0=4, c2=4, c1=4, c0=4,
                        ),
                        axis=X, op=ADD,
                    )

                # ---- PE: 4 S2 matmuls (G), then 16 S1 matmuls (E) ----
                # gp free = 16*c2o + (4*c1o + c0o); rhs j = (c2in, n2)
                gp = ppool.tile([128, 64], dt, tag="gp")
                for c2o in range(4):
                    nc.tensor.matmul(
                        gp[:, c2o * 16:(c2o + 1) * 16],
                        s2sb[:, c2o * 128:(c2o + 1) * 128],
                        A2[:, 0:16],
                        start=True, stop=True,
                    )
                # c1p free = 64*(4*n2o + c2o) + (16*n0o + 4*c1o + c0o)
                c1p = ppool.tile([128, 1024], dt, tag="c1p")
                for n2o in range(4):
                    for c2o in range(4):
                        blk = n2o * 4 + c2o
                        nc.tensor.matmul(
                            c1p[:, blk * 64:(blk + 1) * 64],
                            s1sb[:, blk * 128:(blk + 1) * 128],
                            A[:, 0:64],
                            start=True, stop=True,
                        )

                # ---- assemble in-place, store halves as they finish ----
                c1e = c1p[:].rearrange(
                    "p (n2o c2o n0 cc) -> p n2o c2o n0 cc",
                    n2o=4, c2o=4, n0=4, cc=16,
                )
                xts = xt[:].rearrange(
                    "p (n2 n1 n0 i c2 cc) -> p n2 n1 c2 n0 i cc",
                    n2=4, n1=4, n0=4, i=3, c2=4, cc=16,
                )
                gb = gp[:].rearrange("p (o c) -> p o c", o=1, c=64)
                outv = out[bs].rearrange(
                    "b (ph j) i c -> (b ph) (j i c)", ph=64
                )
                # The j<32 assembly (G bcast + E evacs n2o 0,1) rides ACT
                # alone: the first store trigger then depends on one ACT
                # counter and is immune to the DVE list-scheduler slotting
                # the next group's ladder between evac copies (measured
                # +3.5us on the read->write transition).  j>=32 evacs go to
                # DVE (its store has ring-FIFO slack) - this also frees ACT
                # and DVE to run the two halves concurrently.
                nc.scalar.copy(
                    xtv[:, 0:32, 2, :], gb.broadcast_to((128, 32, 64))
                )
                for n2o in range(2):
                    for n1o in range(4):
                        nc.scalar.copy(
                            xts[:, n2o, n1o, :, :, 1, :],
                            c1e[:, n2o, :, :, :],
                        )
                for n2o in range(2, 4):
                    for n1o in range(4):
                        nc.vector.tensor_copy(
                            xts[:, n2o, n1o, :, :, 1, :],
                            c1e[:, n2o, :, :, :],
                        )
                nc.scalar.copy(
                    xtv[:, 32:64, 2, :], gb.broadcast_to((128, 32, 64))
                )
                stores.append((outv, xt))

            for outv, xt in stores:
                nc.sync.dma_start(outv[:, 0:6144], xt[:, 0:6144])
                nc.sync.dma_start(outv[:, 6144:12288], xt[:, 6144:12288])
    nc.compile()
    return nc


_NC_CACHE = {}


def _get_nc():
    if "nc" not in _NC_CACHE:
        _NC_CACHE["nc"] = _build_nc()
    return _NC_CACHE["nc"]


def kernel(**inputs: np.ndarray) -> np.ndarray:
    import ml_dtypes
    from concourse.bass_utils import run_bass_kernel_spmd

    x = np.ascontiguousarray(inputs["x"], dtype=np.float32)
    assert x.shape == (B_FULL, N, LVL, C), x.shape
    S1, S2 = _make_selectors()
    S12 = np.ascontiguousarray(
        np.concatenate([S1, S2], axis=1).astype(ml_dtypes.bfloat16)
    )
    nc = _get_nc()
    in_maps = [
        {"x": np.ascontiguousarray(x[k * B_CORE:(k + 1) * B_CORE]),
         "s12": S12}
        for k in range(N_CORES)
    ]
    res = run_bass_kernel_spmd(nc, in_maps, list(range(N_CORES)))
    outs = [res.results[k]["out"] for k in range(N_CORES)]
    return np.ascontiguousarray(np.concatenate(outs, axis=0))
